# revision 34
# baseline (speedup 1.0000x reference)
"""Multi-head attention (B=4, S=2048, D=1024, H=16, Dh=64) on 8 TRN2 NeuronCores.

Sharding: core c -> batch b = c//2, head-group g = c%2 (8 heads, output cols
g*512:(g+1)*512).  Host ships x pre-transposed ([D, S]) and weights in bf16;
each core runs attention for its (batch, 8 heads) slice and returns the
TRANSPOSED, UNNORMALIZED per-head outputs [8*65, S] (64 dh rows + 1 softmax
denominator row per head); the host divides by the denominator, transposes,
adds the V bias (softmax rows sum to 1, so attn @ (xWv + bv) = attn @ xWv
+ bv) and concatenates.

Per-core kernel (bf16 compute, f32 accumulation), ScalarE(exp)-bound design:
  - q/k kept natural per head-pair: qT/kT [128, S] with head 2p in rows
    0:64, head 2p+1 in rows 64:128.  Scores run as TWO CONCURRENT K=64
    matmuls (PE row-groups 0/64 via base_partition auto tile_position) into
    the two column halves (= different PSUM banks) of one [128,1024] psc
    tile.  This halves scores PE time vs zero-padded K=128 matmuls.
  - one exp ACTIVATE per psc tile (FD=1024, both heads) -> e bf16; a dummy
    1-element ACT right after the bias DMAs preloads the exp table set
    (~2.7us) off the critical path.
  - v natural = xT.T @ Wv, augmented with a ones-column per head so the AV
    matmul also produces softmax denominators in output row 64.
  - AV: po_h[65, 512] PSUM accumulators (1 bank each) per (head, sq-tile),
    accumulated over the 16 sk chunks; copied (unnormalized, with denom row)
    by DVE to SBUF outT tiles and DMA'd out per sq-tile.  The next tile's
    first three AV groups carry graded due-slack so these copies stay off
    the PE critical path.  No PE transposes, no on-device normalize.
  - PSUM: psc 2x2 banks + po 2x1 + proj 2x1 = 8 banks.
  - startup: v projections run DURING the input DMA window (they only need
    xT+wv, which are ordered first); pair-0 q0/k0 follow; k1-3/q1-3 and all
    later pairs' projections are dripped into the chunk stream at <=2
    matmuls per iteration (bursts detour the PE FIFO and starve the ACT
    stream).  AV for chunk i is emitted AFTER scores for chunk i+1 (carried
    across tile/pair boundaries via a due-queue) so the strict-FIFO PE queue
    never head-of-line blocks the next ACT's input.

Steady state is TensorE-bound (~87% PE busy: 216ns/matmul warm, score pairs
~320ns for two concurrent K=64 matmuls incl. one exposed LDWEIGHTS); the
exp ACTIVATEs (1114ns per [128,1024] chunk) hide underneath.
"""

import numpy as np
import ml_dtypes
from contextlib import ExitStack

import concourse.bass as bass
import concourse.bacc as bacc
import concourse.mybir as mybir
import concourse.tile as tile
from concourse.bass_utils import run_bass_kernel_spmd

F32 = mybir.dt.float32
BF16 = mybir.dt.bfloat16

B, S, D = 4, 2048, 1024
H, DH = 16, 64
N_CORES = 8
HPC = 8          # heads per core
NPAIR = HPC // 2  # head pairs per core = 4
DPC = HPC * DH   # output cols per core = 512
SCALE = 1.0 / 32.0  # 1/sqrt(D)

KD = D // 128    # 8 contraction chunks over d_in
NS = S // 128    # 16 sk chunks
NT = S // 512    # 4 sq tiles of 512
OROWS = HPC * (DH + 1)  # 520 output rows (64 dh + denom per head)

_CACHE = {}


def _build_program():
    nc = bacc.Bacc("TRN2", target_bir_lowering=False, debug=False)

    xt_ext = nc.dram_tensor("xt", [D, S], BF16, kind="ExternalInput").ap()
    wq_ext = nc.dram_tensor("wq", [D, DPC], BF16, kind="ExternalInput").ap()
    wk_ext = nc.dram_tensor("wk", [D, DPC], BF16, kind="ExternalInput").ap()
    wv_ext = nc.dram_tensor("wv", [D, DPC], BF16, kind="ExternalInput").ap()
    bq_ext = nc.dram_tensor("bq", [DPC], F32, kind="ExternalInput").ap()
    bk_ext = nc.dram_tensor("bk", [DPC], F32, kind="ExternalInput").ap()
    out_ext = nc.dram_tensor("out", [OROWS, S], BF16, kind="ExternalOutput").ap()

    with tile.TileContext(nc, pool_alloc_mode="queue") as tc, ExitStack() as ctx:
        singles = ctx.enter_context(tc.tile_pool(name="singles", bufs=1))

        # --- DMAs: tiny bias vectors first, then x + v weights (v
        # projections overlap the DMA window), then q/k weights ---
        bq_col = []
        bk_col = []
        for m in range(NPAIR):
            t = singles.tile([128, 1], F32, tag=f"bq{m}", name=f"bq{m}")
            nc.sync.dma_start(
                out=t, in_=bq_ext[m * 128:(m + 1) * 128].rearrange("(p o) -> p o", o=1)
            )
            bq_col.append(t)
            t = singles.tile([128, 1], F32, tag=f"bk{m}", name=f"bk{m}")
            nc.scalar.dma_start(
                out=t, in_=bk_ext[m * 128:(m + 1) * 128].rearrange("(p o) -> p o", o=1)
            )
            bk_col.append(t)

        xT = [singles.tile([128, S], BF16, tag=f"xT{j}", name=f"xT{j}") for j in range(KD)]
        w_bf = {n: [None] * KD for n in ("wq", "wk", "wv")}

        def dma_w(name, ext, k, eng):
            wb = singles.tile([128, DPC], BF16, tag=f"{name}_bf{k}", name=f"{name}_bf{k}")
            eng.dma_start(out=wb, in_=ext[k * 128:(k + 1) * 128, :])
            w_bf[name][k] = wb

        for j in range(KD):
            # x chunk split across both HWDGE rings (SP + ACT) -- a single
            # ring's descriptor dispatch caps DMA throughput; wv rides along
            nc.sync.dma_start(out=xT[j][:, 0:S // 2], in_=xt_ext[j * 128:(j + 1) * 128, 0:S // 2])
            nc.scalar.dma_start(out=xT[j][:, S // 2:], in_=xt_ext[j * 128:(j + 1) * 128, S // 2:])
            dma_w("wv", wv_ext, j, nc.sync if j % 2 == 0 else nc.scalar)
        # wq/wk: only the pair-0 columns gate the first scores -- they ride
        # the fast HW rings right after x+wv; the other three pairs' columns
        # (not read until pair-1's dripped projections, ~60us in) go on the
        # otherwise-idle GPSIMD SWDGE ring, shrinking the critical DMA
        # window by ~1.5MB
        w_ext = {"wq": wq_ext, "wk": wk_ext}
        for j in range(KD):
            for name in ("wq", "wk"):
                wb = singles.tile([128, DPC], BF16, tag=f"{name}_bf{j}", name=f"{name}_bf{j}")
                w_bf[name][j] = wb
            nc.sync.dma_start(out=w_bf["wq"][j][:, 0:128], in_=wq_ext[j * 128:(j + 1) * 128, 0:128])
            nc.scalar.dma_start(out=w_bf["wk"][j][:, 0:128], in_=wk_ext[j * 128:(j + 1) * 128, 0:128])
        for j in range(KD):
            for name in ("wq", "wk"):
                nc.gpsimd.dma_start(
                    out=w_bf[name][j][:, 128:DPC],
                    in_=w_ext[name][j * 128:(j + 1) * 128, 128:DPC],
                )

        # --- persistent sbuf tensors ---
        qT = [singles.tile([128, S], BF16, tag=f"qT{m}", name=f"qT{m}") for m in range(NPAIR)]
        kT = [singles.tile([128, S], BF16, tag=f"kT{m}", name=f"kT{m}") for m in range(NPAIR)]
        vsb = [singles.tile([128, HPC, DH + 1], BF16, tag=f"v{i}", name=f"v{i}") for i in range(NS)]
        outT = [singles.tile([DH + 1, S], BF16, tag=f"oT{h}", name=f"oT{h}") for h in range(HPC)]

        # preload the exp table set while DMAs stream
        dummy = singles.tile([1, 1], BF16, tag="dummy")
        nc.scalar.activation(
            dummy, bq_col[0][0:1, :], mybir.ActivationFunctionType.Exp, scale=SCALE
        )

        # dense warm-up matmuls on a zeroed tile with no DMA dependencies:
        # they bridge PE activity from ~1us until the first v-projection
        # matmuls (~14us) so the HAM clock gate reaches and keeps 2.4GHz --
        # without them the DMA-paced v trickle never fills the activity
        # window and the whole startup runs at 1.2GHz until ~33us.
        junk = singles.tile([128, 512], BF16, tag="junk")
        nc.vector.memset(junk, 0.0)

        # --- psum pools: scores 2x[128,1024]f32 (4 banks) + AV accumulators
        # 2x[65,512] (2 banks) + projection accumulators 2x[128,512] (2) ---
        s_psum = ctx.enter_context(tc.tile_pool(name="s_psum", bufs=2, space="PSUM"))
        o_psum = ctx.enter_context(tc.tile_pool(name="o_psum", bufs=2, space="PSUM"))
        p_psum = ctx.enter_context(tc.tile_pool(name="p_psum", bufs=2, space="PSUM"))

        def gen_1_proj(w, m, n, dst, bias):
            """Generator: one of q/k projection for pair m, sq quarter n."""
            sl = slice(n * 512, (n + 1) * 512)
            ps = p_psum.tile([128, 512], F32, tag="pp", name=f"pp{w}{m}_{n}")
            for k in range(KD):
                nc.tensor.matmul(
                    ps,
                    lhsT=w_bf[w][k][:, m * 128:(m + 1) * 128],
                    rhs=xT[k][:, sl],
                    start=(k == 0),
                    stop=(k == KD - 1),
                )
                if k % 2 == 1:
                    yield
            nc.vector.tensor_scalar_add(dst[m][:, sl], ps, bias[m])

        def gen_q_proj(m, n):
            return gen_1_proj("wq", m, n, qT, bq_col)

        def gen_k_proj(m, n):
            return gen_1_proj("wk", m, n, kT, bk_col)

        def emit(gen):
            for _ in gen:
                pass

        def gen_v_proj(i):
            """Generator: v projection for sk chunk i (no bias -- host adds
            bv after normalize since softmax rows sum to 1)."""
            ps = p_psum.tile([128, 512], F32, tag="pp", name=f"vp{i}")
            for k in range(KD):
                nc.tensor.matmul(
                    ps,
                    lhsT=xT[k][:, i * 128:(i + 1) * 128],
                    rhs=w_bf["wv"][k],
                    start=(k == 0),
                    stop=(k == KD - 1),
                )
                if k % 2 == 1:
                    yield
            nc.vector.tensor_copy(
                vsb[i][:, :, 0:DH], ps.rearrange("p (h d) -> p h d", h=HPC)
            )
            nc.vector.memset(vsb[i][:, :, DH:DH + 1], 1.0)

        warm = p_psum.tile([128, 512], F32, tag="pp", name="warm")
        for i in range(48):
            nc.tensor.matmul(
                warm, lhsT=junk[:, 0:128], rhs=junk, start=True, stop=True
            )

        # only the first two v projections and pair-0's first-quarter q/k
        # run before the chunk stream; v(2..15) and the other pair-0
        # quarters drip into tile 0's PE slack (the whole v stream before
        # the first score serialized ~20us of PE work behind the DMA tail)
        emit(gen_v_proj(0))
        emit(gen_v_proj(1))
        emit(gen_q_proj(0, 0))
        emit(gen_k_proj(0, 0))

        e_pool = ctx.enter_context(tc.tile_pool(name="e_pool", bufs=10))

        # filler[g] = list of steps (2 proj matmuls each) to emit at global
        # iteration g.  Deadlines: pair-0 k quarter n is needed by t=0
        # chunk c=4n; pair-0 q quarter n by tile t=n; pair p>0 fully by its
        # pair start (dripped uniformly over pair p-1's 64 iterations).
        n_glob = NPAIR * NT * NS
        filler = [[] for _ in range(n_glob + 1)]

        def schedule(gen, g0, g1, nsteps=5):
            """Spread `nsteps` consumption steps of gen over [g0, g1)."""
            slots = np.linspace(g0, g1, nsteps, endpoint=False).astype(int)
            for s in slots[:-1]:
                filler[s].append(gen.__next__)
            filler[slots[-1]].append(lambda gg=gen: list(gg))

        def atomic(gen, slot):
            """Emit a whole projection unit at one slot.  Tile-0 fillers
            must be atomic: interleaving two open generators plus a third
            allocation on the 2-buffer proj-psum tag would recycle a buffer
            mid-accumulation."""
            filler[slot].append(lambda gg=gen: list(gg))

        for c in range(2, NS):
            atomic(gen_v_proj(c), c - 2)
        atomic(gen_k_proj(0, 1), 0)
        atomic(gen_k_proj(0, 2), 3)
        atomic(gen_k_proj(0, 3), 6)
        atomic(gen_q_proj(0, 1), 10)
        # pair-0's q2/q3 and all of pair-1's units interleave evenly over
        # tiles 1-3 (separate overlapping windows bunched 4 matmuls into
        # single iterations and starved the exp stream); deadlines: q2 by
        # g=32, q3 by g=48, pair-1 fully by g=64
        units01 = [
            gen_q_proj(1, 0), gen_k_proj(1, 0), gen_q_proj(0, 2),
            gen_k_proj(1, 1), gen_k_proj(1, 2), gen_q_proj(0, 3),
            gen_k_proj(1, 3), gen_q_proj(1, 1), gen_q_proj(1, 2),
            gen_q_proj(1, 3),
        ]
        for u, g in enumerate(units01):
            schedule(g, 16 + (u * 48) // 10, 16 + ((u + 1) * 48) // 10, 5)
        for p in range(2, NPAIR):
            base = (p - 1) * NT * NS
            units = [gen_q_proj(p, 0), gen_k_proj(p, 0)]
            for n in range(1, NT):
                units.append(gen_k_proj(p, n))
            for n in range(1, NT):
                units.append(gen_q_proj(p, n))
            for u, g in enumerate(units):
                schedule(g, base + 2 + u * 8, base + 2 + (u + 1) * 8, 5)

        # iterations are processed in groups of 2 chunks (fewer pending-
        # queue boundaries; the paired emission also keeps the psc pipeline
        # regular).  AVs trail their chunk by one group so the strict-FIFO
        # PE queue always has the next chunk's score matmuls ahead of any
        # instruction that waits on an exp result.
        pending = []  # (due_group, seq, fn): AVs, copies, DMAs
        seq_ctr = 0

        def push(due, fn):
            nonlocal seq_ctr
            pending.append((due, seq_ctr, fn))
            seq_ctr += 1

        def emit_exp(hp, t, c, psc):
            e = e_pool.tile([128, 1024], BF16, tag="e", name=f"e{hp}_{t}_{c}")
            nc.scalar.activation(
                e, psc, mybir.ActivationFunctionType.Exp, scale=SCALE
            )
            return e

        for g0 in range(0, n_glob, 2):
            G = g0 // 2
            hp, r = divmod(g0, NT * NS)
            t, c0 = divmod(r, NS)
            h0, h1 = 2 * hp, 2 * hp + 1
            tsl = slice(t * 512, (t + 1) * 512)
            if c0 == 0:
                po_h = o_psum.tile([DH + 1, 512], F32, tag="po", name=f"po{hp}_{t}a")
                po_h1 = o_psum.tile([DH + 1, 512], F32, tag="po", name=f"po{hp}_{t}b")
            pscs = []
            for c in (c0, c0 + 1):
                psc = s_psum.tile([128, 1024], F32, tag="psc", name=f"ps{hp}_{t}_{c}")
                pscs.append(psc)
                nc.tensor.matmul(
                    psc[:, 0:512],
                    lhsT=kT[hp][0:64, c * 128:(c + 1) * 128],
                    rhs=qT[hp][0:64, tsl],
                    start=True, stop=True,
                )
                nc.tensor.matmul(
                    psc[:, 512:1024],
                    lhsT=kT[hp][64:128, c * 128:(c + 1) * 128],
                    rhs=qT[hp][64:128, tsl],
                    start=True, stop=True,
                )
            due_now = sorted([p for p in pending if p[0] <= G], key=lambda p: p[1])
            pending = [p for p in pending if p[0] > G]
            for _, _, fn in due_now:
                fn()
            for g in (g0, g0 + 1):
                for fn in filler[g]:
                    fn()
                filler[g] = []
            for i, c in enumerate((c0, c0 + 1)):
                e = emit_exp(hp, t, c, pscs[i])

                def av(c=c, e=e, po_h=po_h, po_h1=po_h1, h0=h0, h1=h1):
                    nc.tensor.matmul(
                        po_h, lhsT=vsb[c][:, h0, :], rhs=e[:, 0:512],
                        start=(c == 0), stop=(c == NS - 1),
                    )
                    nc.tensor.matmul(
                        po_h1, lhsT=vsb[c][:, h1, :], rhs=e[:, 512:1024],
                        start=(c == 0), stop=(c == NS - 1),
                    )
                # the tile's first three AV groups get graded extra slack
                # (all landing at the same absolute group, so psum write
                # order stays c-ascending): the boundary po copies then
                # never sit on the PE's critical path
                gl = c0 // 2
                push(G + (3 if gl == 0 else 2 if gl == 1 else 1), av)
            if c0 + 1 == NS - 1:
                def tail(t=t, po_h=po_h, po_h1=po_h1, h0=h0, h1=h1):
                    # DVE-only copies: the deep AV slack above hides their
                    # latency, and keeping them off ScalarE avoids pausing
                    # the exp stream
                    tsl = slice(t * 512, (t + 1) * 512)
                    for h, po in ((h0, po_h), (h1, po_h1)):
                        nc.vector.tensor_copy(outT[h][:, tsl], po)
                        nc.sync.dma_start(
                            out=out_ext[h * (DH + 1):(h + 1) * (DH + 1), tsl],
                            in_=outT[h][:, tsl],
                        )
                push(G + 1, tail)
        for _, _, fn in sorted(pending, key=lambda p: p[1]):
            fn()
        for fns in filler:
            for fn in fns:
                fn()

    nc.compile()
    return nc


def _get_program():
    if "nc" not in _CACHE:
        _CACHE["nc"] = _build_program()
    return _CACHE["nc"]


def kernel(x, Wq, bq, Wk, bk, Wv, bv, _trace=False):
    bf = ml_dtypes.bfloat16
    x = np.asarray(x, dtype=np.float32)
    Wq = np.asarray(Wq, dtype=np.float32)
    Wk = np.asarray(Wk, dtype=np.float32)
    Wv = np.asarray(Wv, dtype=np.float32)
    bq = np.ascontiguousarray(np.asarray(bq, dtype=np.float32))
    bk = np.ascontiguousarray(np.asarray(bk, dtype=np.float32))
    bv = np.asarray(bv, dtype=np.float32)

    nc = _get_program()

    in_maps = []
    for c in range(N_CORES):
        b, g = c // 2, c % 2
        cols = slice(g * DPC, (g + 1) * DPC)
        in_maps.append(
            {
                "xt": np.ascontiguousarray(x[b].T.astype(bf)),
                "wq": np.ascontiguousarray(Wq[:, cols].astype(bf)),
                "wk": np.ascontiguousarray(Wk[:, cols].astype(bf)),
                "wv": np.ascontiguousarray(Wv[:, cols].astype(bf)),
                "bq": np.ascontiguousarray(bq[cols]),
                "bk": np.ascontiguousarray(bk[cols]),
            }
        )

    res = run_bass_kernel_spmd(nc, in_maps, core_ids=list(range(N_CORES)), trace=_trace)
    _CACHE["last_results"] = res

    out = np.empty((B, S, D), dtype=np.float32)
    for c in range(N_CORES):
        b, g = c // 2, c % 2
        o = res.results[c]["out"].astype(np.float32).reshape(HPC, DH + 1, S)
        nrm = o[:, 0:DH, :] / o[:, DH:DH + 1, :]      # [8, 64, S]
        out[b, :, g * DPC:(g + 1) * DPC] = nrm.transpose(2, 0, 1).reshape(S, DPC)
    out += bv
    return out


# revision 35
# speedup vs baseline: 1.0193x; 1.0193x over previous
"""Multi-head attention (B=4, S=2048, D=1024, H=16, Dh=64) on 8 TRN2 NeuronCores.

Sharding: core c -> batch b = c//2, head-group g = c%2 (8 heads, output cols
g*512:(g+1)*512).  Host ships x pre-transposed ([D, S]) and weights in bf16;
each core runs attention for its (batch, 8 heads) slice and returns the
TRANSPOSED, UNNORMALIZED per-head outputs [8*65, S] (64 dh rows + 1 softmax
denominator row per head); the host divides by the denominator, transposes,
adds the V bias (softmax rows sum to 1, so attn @ (xWv + bv) = attn @ xWv
+ bv) and concatenates.

Per-core kernel (bf16 compute, f32 accumulation), ScalarE(exp)-bound design:
  - q/k kept natural per head-pair: qT/kT [128, S] with head 2p in rows
    0:64, head 2p+1 in rows 64:128.  Scores run as TWO CONCURRENT K=64
    matmuls (PE row-groups 0/64 via base_partition auto tile_position) into
    the two column halves (= different PSUM banks) of one [128,1024] psc
    tile.  This halves scores PE time vs zero-padded K=128 matmuls.
  - one exp ACTIVATE per psc tile (FD=1024, both heads) -> e bf16; a dummy
    1-element ACT right after the bias DMAs preloads the exp table set
    (~2.7us) off the critical path.
  - v natural = xT.T @ Wv, augmented with a ones-column per head so the AV
    matmul also produces softmax denominators in output row 64.
  - AV: po_h[65, 512] PSUM accumulators (1 bank each) per (head, sq-tile),
    accumulated over the 16 sk chunks; copied (unnormalized, with denom row)
    by DVE to SBUF outT tiles and DMA'd out per sq-tile.  The next tile's
    first three AV groups carry graded due-slack so these copies stay off
    the PE critical path.  No PE transposes, no on-device normalize.
  - PSUM: psc 2x2 banks + po 2x1 + proj 2x1 = 8 banks.
  - startup: v projections run DURING the input DMA window (they only need
    xT+wv, which are ordered first); pair-0 q0/k0 follow; k1-3/q1-3 and all
    later pairs' projections are dripped into the chunk stream at <=2
    matmuls per iteration (bursts detour the PE FIFO and starve the ACT
    stream).  AV for chunk i is emitted AFTER scores for chunk i+1 (carried
    across tile/pair boundaries via a due-queue) so the strict-FIFO PE queue
    never head-of-line blocks the next ACT's input.

Steady state is TensorE-bound (~87% PE busy: 216ns/matmul warm, score pairs
~320ns for two concurrent K=64 matmuls incl. one exposed LDWEIGHTS); the
exp ACTIVATEs (1114ns per [128,1024] chunk) hide underneath.
"""

import numpy as np
import ml_dtypes
from contextlib import ExitStack

import concourse.bass as bass
import concourse.bacc as bacc
import concourse.mybir as mybir
import concourse.tile as tile
from concourse.bass_utils import run_bass_kernel_spmd

F32 = mybir.dt.float32
BF16 = mybir.dt.bfloat16

B, S, D = 4, 2048, 1024
H, DH = 16, 64
N_CORES = 8
HPC = 8          # heads per core
NPAIR = HPC // 2  # head pairs per core = 4
DPC = HPC * DH   # output cols per core = 512
SCALE = 1.0 / 32.0  # 1/sqrt(D)

KD = D // 128    # 8 contraction chunks over d_in
NS = S // 128    # 16 sk chunks
NT = S // 512    # 4 sq tiles of 512
OROWS = HPC * (DH + 1)  # 520 output rows (64 dh + denom per head)

_CACHE = {}


def _build_program():
    nc = bacc.Bacc("TRN2", target_bir_lowering=False, debug=False)

    xt_ext = nc.dram_tensor("xt", [D, S], BF16, kind="ExternalInput").ap()
    wq_ext = nc.dram_tensor("wq", [D, DPC], BF16, kind="ExternalInput").ap()
    wk_ext = nc.dram_tensor("wk", [D, DPC], BF16, kind="ExternalInput").ap()
    wv_ext = nc.dram_tensor("wv", [D, DPC], BF16, kind="ExternalInput").ap()
    bq_ext = nc.dram_tensor("bq", [DPC], F32, kind="ExternalInput").ap()
    bk_ext = nc.dram_tensor("bk", [DPC], F32, kind="ExternalInput").ap()
    out_ext = nc.dram_tensor("out", [OROWS, S], BF16, kind="ExternalOutput").ap()

    with tile.TileContext(nc, pool_alloc_mode="queue") as tc, ExitStack() as ctx:
        singles = ctx.enter_context(tc.tile_pool(name="singles", bufs=1))

        # --- DMAs: tiny bias vectors first, then x + v weights (v
        # projections overlap the DMA window), then q/k weights ---
        bq_col = []
        bk_col = []
        for m in range(NPAIR):
            t = singles.tile([128, 1], F32, tag=f"bq{m}", name=f"bq{m}")
            nc.sync.dma_start(
                out=t, in_=bq_ext[m * 128:(m + 1) * 128].rearrange("(p o) -> p o", o=1)
            )
            bq_col.append(t)
            t = singles.tile([128, 1], F32, tag=f"bk{m}", name=f"bk{m}")
            nc.scalar.dma_start(
                out=t, in_=bk_ext[m * 128:(m + 1) * 128].rearrange("(p o) -> p o", o=1)
            )
            bk_col.append(t)

        xT = [singles.tile([128, S], BF16, tag=f"xT{j}", name=f"xT{j}") for j in range(KD)]
        w_bf = {n: [None] * KD for n in ("wq", "wk", "wv")}

        def dma_w(name, ext, k, eng):
            wb = singles.tile([128, DPC], BF16, tag=f"{name}_bf{k}", name=f"{name}_bf{k}")
            eng.dma_start(out=wb, in_=ext[k * 128:(k + 1) * 128, :])
            w_bf[name][k] = wb

        for j in range(KD):
            # x chunk split across both HWDGE rings (SP + ACT) -- a single
            # ring's descriptor dispatch caps DMA throughput; wv rides along
            nc.sync.dma_start(out=xT[j][:, 0:S // 2], in_=xt_ext[j * 128:(j + 1) * 128, 0:S // 2])
            nc.scalar.dma_start(out=xT[j][:, S // 2:], in_=xt_ext[j * 128:(j + 1) * 128, S // 2:])
            dma_w("wv", wv_ext, j, nc.sync if j % 2 == 0 else nc.scalar)
        # wq/wk in two passes on the HW rings: only the pair-0 columns gate
        # the first scores, so they ride right after x+wv; the other three
        # pairs' columns (not read until pair-1's dripped projections,
        # ~60us in) follow once the critical window is over.  (A SWDGE
        # side-channel for the rest was tried and hurt: it steals HBM
        # bandwidth from the rings during the critical x window.)
        w_ext = {"wq": wq_ext, "wk": wk_ext}
        for j in range(KD):
            for name in ("wq", "wk"):
                wb = singles.tile([128, DPC], BF16, tag=f"{name}_bf{j}", name=f"{name}_bf{j}")
                w_bf[name][j] = wb
            nc.sync.dma_start(out=w_bf["wq"][j][:, 0:128], in_=wq_ext[j * 128:(j + 1) * 128, 0:128])
            nc.scalar.dma_start(out=w_bf["wk"][j][:, 0:128], in_=wk_ext[j * 128:(j + 1) * 128, 0:128])
        for j in range(KD):
            for i, name in enumerate(("wq", "wk")):
                eng = nc.sync if (j + i) % 2 == 0 else nc.scalar
                eng.dma_start(
                    out=w_bf[name][j][:, 128:DPC],
                    in_=w_ext[name][j * 128:(j + 1) * 128, 128:DPC],
                )

        # --- persistent sbuf tensors ---
        qT = [singles.tile([128, S], BF16, tag=f"qT{m}", name=f"qT{m}") for m in range(NPAIR)]
        kT = [singles.tile([128, S], BF16, tag=f"kT{m}", name=f"kT{m}") for m in range(NPAIR)]
        vsb = [singles.tile([128, HPC, DH + 1], BF16, tag=f"v{i}", name=f"v{i}") for i in range(NS)]
        outT = [singles.tile([DH + 1, S], BF16, tag=f"oT{h}", name=f"oT{h}") for h in range(HPC)]

        # preload the exp table set while DMAs stream
        dummy = singles.tile([1, 1], BF16, tag="dummy")
        nc.scalar.activation(
            dummy, bq_col[0][0:1, :], mybir.ActivationFunctionType.Exp, scale=SCALE
        )

        # dense warm-up matmuls on a zeroed tile with no DMA dependencies:
        # they bridge PE activity from ~1us until the first v-projection
        # matmuls (~14us) so the HAM clock gate reaches and keeps 2.4GHz --
        # without them the DMA-paced v trickle never fills the activity
        # window and the whole startup runs at 1.2GHz until ~33us.
        junk = singles.tile([128, 512], BF16, tag="junk")
        nc.vector.memset(junk, 0.0)

        # --- psum pools: scores 2x[128,1024]f32 (4 banks) + AV accumulators
        # 2x[65,512] (2 banks) + projection accumulators 2x[128,512] (2) ---
        s_psum = ctx.enter_context(tc.tile_pool(name="s_psum", bufs=2, space="PSUM"))
        o_psum = ctx.enter_context(tc.tile_pool(name="o_psum", bufs=2, space="PSUM"))
        p_psum = ctx.enter_context(tc.tile_pool(name="p_psum", bufs=2, space="PSUM"))

        def gen_1_proj(w, m, n, dst, bias):
            """Generator: one of q/k projection for pair m, sq quarter n."""
            sl = slice(n * 512, (n + 1) * 512)
            ps = p_psum.tile([128, 512], F32, tag="pp", name=f"pp{w}{m}_{n}")
            for k in range(KD):
                nc.tensor.matmul(
                    ps,
                    lhsT=w_bf[w][k][:, m * 128:(m + 1) * 128],
                    rhs=xT[k][:, sl],
                    start=(k == 0),
                    stop=(k == KD - 1),
                )
                if k % 2 == 1:
                    yield
            nc.vector.tensor_scalar_add(dst[m][:, sl], ps, bias[m])

        def gen_q_proj(m, n):
            return gen_1_proj("wq", m, n, qT, bq_col)

        def gen_k_proj(m, n):
            return gen_1_proj("wk", m, n, kT, bk_col)

        def emit(gen):
            for _ in gen:
                pass

        def gen_v_proj(i):
            """Generator: v projection for sk chunk i (no bias -- host adds
            bv after normalize since softmax rows sum to 1)."""
            ps = p_psum.tile([128, 512], F32, tag="pp", name=f"vp{i}")
            for k in range(KD):
                nc.tensor.matmul(
                    ps,
                    lhsT=xT[k][:, i * 128:(i + 1) * 128],
                    rhs=w_bf["wv"][k],
                    start=(k == 0),
                    stop=(k == KD - 1),
                )
                if k % 2 == 1:
                    yield
            nc.vector.tensor_copy(
                vsb[i][:, :, 0:DH], ps.rearrange("p (h d) -> p h d", h=HPC)
            )
            nc.vector.memset(vsb[i][:, :, DH:DH + 1], 1.0)

        warm = p_psum.tile([128, 512], F32, tag="pp", name="warm")
        for i in range(48):
            nc.tensor.matmul(
                warm, lhsT=junk[:, 0:128], rhs=junk, start=True, stop=True
            )

        # only the first two v projections and pair-0's first-quarter q/k
        # run before the chunk stream; v(2..15) and the other pair-0
        # quarters drip into tile 0's PE slack (the whole v stream before
        # the first score serialized ~20us of PE work behind the DMA tail)
        emit(gen_v_proj(0))
        emit(gen_v_proj(1))
        emit(gen_q_proj(0, 0))
        emit(gen_k_proj(0, 0))

        e_pool = ctx.enter_context(tc.tile_pool(name="e_pool", bufs=10))

        # filler[g] = list of steps (2 proj matmuls each) to emit at global
        # iteration g.  Deadlines: pair-0 k quarter n is needed by t=0
        # chunk c=4n; pair-0 q quarter n by tile t=n; pair p>0 fully by its
        # pair start (dripped uniformly over pair p-1's 64 iterations).
        n_glob = NPAIR * NT * NS
        filler = [[] for _ in range(n_glob + 1)]

        def schedule(gen, g0, g1, nsteps=5):
            """Spread `nsteps` consumption steps of gen over [g0, g1)."""
            slots = np.linspace(g0, g1, nsteps, endpoint=False).astype(int)
            for s in slots[:-1]:
                filler[s].append(gen.__next__)
            filler[slots[-1]].append(lambda gg=gen: list(gg))

        def atomic(gen, slot):
            """Emit a whole projection unit at one slot.  Tile-0 fillers
            must be atomic: interleaving two open generators plus a third
            allocation on the 2-buffer proj-psum tag would recycle a buffer
            mid-accumulation."""
            filler[slot].append(lambda gg=gen: list(gg))

        for c in range(2, NS):
            atomic(gen_v_proj(c), c - 2)
        atomic(gen_k_proj(0, 1), 0)
        atomic(gen_k_proj(0, 2), 3)
        atomic(gen_k_proj(0, 3), 6)
        atomic(gen_q_proj(0, 1), 10)
        # pair-0's q2/q3 and all of pair-1's units interleave evenly over
        # tiles 1-3 (separate overlapping windows bunched 4 matmuls into
        # single iterations and starved the exp stream); deadlines: q2 by
        # g=32, q3 by g=48, pair-1 fully by g=64
        units01 = [
            gen_q_proj(1, 0), gen_k_proj(1, 0), gen_q_proj(0, 2),
            gen_k_proj(1, 1), gen_k_proj(1, 2), gen_q_proj(0, 3),
            gen_k_proj(1, 3), gen_q_proj(1, 1), gen_q_proj(1, 2),
            gen_q_proj(1, 3),
        ]
        for u, g in enumerate(units01):
            schedule(g, 16 + (u * 48) // 10, 16 + ((u + 1) * 48) // 10, 5)
        for p in range(2, NPAIR):
            base = (p - 1) * NT * NS
            units = [gen_q_proj(p, 0), gen_k_proj(p, 0)]
            for n in range(1, NT):
                units.append(gen_k_proj(p, n))
            for n in range(1, NT):
                units.append(gen_q_proj(p, n))
            for u, g in enumerate(units):
                schedule(g, base + 2 + u * 8, base + 2 + (u + 1) * 8, 5)

        # iterations are processed in groups of 2 chunks (fewer pending-
        # queue boundaries; the paired emission also keeps the psc pipeline
        # regular).  AVs trail their chunk by one group so the strict-FIFO
        # PE queue always has the next chunk's score matmuls ahead of any
        # instruction that waits on an exp result.
        pending = []  # (due_group, seq, fn): AVs, copies, DMAs
        seq_ctr = 0

        def push(due, fn):
            nonlocal seq_ctr
            pending.append((due, seq_ctr, fn))
            seq_ctr += 1

        def emit_exp(hp, t, c, psc):
            e = e_pool.tile([128, 1024], BF16, tag="e", name=f"e{hp}_{t}_{c}")
            nc.scalar.activation(
                e, psc, mybir.ActivationFunctionType.Exp, scale=SCALE
            )
            return e

        for g0 in range(0, n_glob, 2):
            G = g0 // 2
            hp, r = divmod(g0, NT * NS)
            t, c0 = divmod(r, NS)
            h0, h1 = 2 * hp, 2 * hp + 1
            tsl = slice(t * 512, (t + 1) * 512)
            if c0 == 0:
                po_h = o_psum.tile([DH + 1, 512], F32, tag="po", name=f"po{hp}_{t}a")
                po_h1 = o_psum.tile([DH + 1, 512], F32, tag="po", name=f"po{hp}_{t}b")
            pscs = []
            for c in (c0, c0 + 1):
                psc = s_psum.tile([128, 1024], F32, tag="psc", name=f"ps{hp}_{t}_{c}")
                pscs.append(psc)
                nc.tensor.matmul(
                    psc[:, 0:512],
                    lhsT=kT[hp][0:64, c * 128:(c + 1) * 128],
                    rhs=qT[hp][0:64, tsl],
                    start=True, stop=True,
                )
                nc.tensor.matmul(
                    psc[:, 512:1024],
                    lhsT=kT[hp][64:128, c * 128:(c + 1) * 128],
                    rhs=qT[hp][64:128, tsl],
                    start=True, stop=True,
                )
            due_now = sorted([p for p in pending if p[0] <= G], key=lambda p: p[1])
            pending = [p for p in pending if p[0] > G]
            for _, _, fn in due_now:
                fn()
            for g in (g0, g0 + 1):
                for fn in filler[g]:
                    fn()
                filler[g] = []
            for i, c in enumerate((c0, c0 + 1)):
                e = emit_exp(hp, t, c, pscs[i])

                def av(c=c, e=e, po_h=po_h, po_h1=po_h1, h0=h0, h1=h1):
                    nc.tensor.matmul(
                        po_h, lhsT=vsb[c][:, h0, :], rhs=e[:, 0:512],
                        start=(c == 0), stop=(c == NS - 1),
                    )
                    nc.tensor.matmul(
                        po_h1, lhsT=vsb[c][:, h1, :], rhs=e[:, 512:1024],
                        start=(c == 0), stop=(c == NS - 1),
                    )
                # the tile's first three AV groups get graded extra slack
                # (all landing at the same absolute group, so psum write
                # order stays c-ascending): the boundary po copies then
                # never sit on the PE's critical path
                gl = c0 // 2
                push(G + (3 if gl == 0 else 2 if gl == 1 else 1), av)
            if c0 + 1 == NS - 1:
                def tail(t=t, po_h=po_h, po_h1=po_h1, h0=h0, h1=h1):
                    # DVE-only copies: the deep AV slack above hides their
                    # latency, and keeping them off ScalarE avoids pausing
                    # the exp stream
                    tsl = slice(t * 512, (t + 1) * 512)
                    for h, po in ((h0, po_h), (h1, po_h1)):
                        nc.vector.tensor_copy(outT[h][:, tsl], po)
                        nc.sync.dma_start(
                            out=out_ext[h * (DH + 1):(h + 1) * (DH + 1), tsl],
                            in_=outT[h][:, tsl],
                        )
                push(G + 1, tail)
        for _, _, fn in sorted(pending, key=lambda p: p[1]):
            fn()
        for fns in filler:
            for fn in fns:
                fn()

    nc.compile()
    return nc


def _get_program():
    if "nc" not in _CACHE:
        _CACHE["nc"] = _build_program()
    return _CACHE["nc"]


def kernel(x, Wq, bq, Wk, bk, Wv, bv, _trace=False):
    bf = ml_dtypes.bfloat16
    x = np.asarray(x, dtype=np.float32)
    Wq = np.asarray(Wq, dtype=np.float32)
    Wk = np.asarray(Wk, dtype=np.float32)
    Wv = np.asarray(Wv, dtype=np.float32)
    bq = np.ascontiguousarray(np.asarray(bq, dtype=np.float32))
    bk = np.ascontiguousarray(np.asarray(bk, dtype=np.float32))
    bv = np.asarray(bv, dtype=np.float32)

    nc = _get_program()

    in_maps = []
    for c in range(N_CORES):
        b, g = c // 2, c % 2
        cols = slice(g * DPC, (g + 1) * DPC)
        in_maps.append(
            {
                "xt": np.ascontiguousarray(x[b].T.astype(bf)),
                "wq": np.ascontiguousarray(Wq[:, cols].astype(bf)),
                "wk": np.ascontiguousarray(Wk[:, cols].astype(bf)),
                "wv": np.ascontiguousarray(Wv[:, cols].astype(bf)),
                "bq": np.ascontiguousarray(bq[cols]),
                "bk": np.ascontiguousarray(bk[cols]),
            }
        )

    res = run_bass_kernel_spmd(nc, in_maps, core_ids=list(range(N_CORES)), trace=_trace)
    _CACHE["last_results"] = res

    out = np.empty((B, S, D), dtype=np.float32)
    for c in range(N_CORES):
        b, g = c // 2, c % 2
        o = res.results[c]["out"].astype(np.float32).reshape(HPC, DH + 1, S)
        nrm = o[:, 0:DH, :] / o[:, DH:DH + 1, :]      # [8, 64, S]
        out[b, :, g * DPC:(g + 1) * DPC] = nrm.transpose(2, 0, 1).reshape(S, DPC)
    out += bv
    return out


# revision 36
# speedup vs baseline: 1.0222x; 1.0029x over previous
"""Multi-head attention (B=4, S=2048, D=1024, H=16, Dh=64) on 8 TRN2 NeuronCores.

Sharding: core c -> batch b = c//2, head-group g = c%2 (8 heads, output cols
g*512:(g+1)*512).  Host ships x pre-transposed ([D, S]) and weights in bf16;
each core runs attention for its (batch, 8 heads) slice and returns the
TRANSPOSED, UNNORMALIZED per-head outputs [8*65, S] (64 dh rows + 1 softmax
denominator row per head); the host divides by the denominator, transposes,
adds the V bias (softmax rows sum to 1, so attn @ (xWv + bv) = attn @ xWv
+ bv) and concatenates.

Per-core kernel (bf16 compute, f32 accumulation), ScalarE(exp)-bound design:
  - q/k kept natural per head-pair: qT/kT [128, S] with head 2p in rows
    0:64, head 2p+1 in rows 64:128.  Scores run as TWO CONCURRENT K=64
    matmuls (PE row-groups 0/64 via base_partition auto tile_position) into
    the two column halves (= different PSUM banks) of one [128,1024] psc
    tile.  This halves scores PE time vs zero-padded K=128 matmuls.
  - one exp ACTIVATE per psc tile (FD=1024, both heads) -> e bf16; a dummy
    1-element ACT right after the bias DMAs preloads the exp table set
    (~2.7us) off the critical path.
  - v natural = xT.T @ Wv, augmented with a ones-column per head so the AV
    matmul also produces softmax denominators in output row 64.
  - AV: po_h[65, 512] PSUM accumulators (1 bank each) per (head, sq-tile),
    accumulated over the 16 sk chunks; copied (unnormalized, with denom row)
    by DVE to SBUF outT tiles and DMA'd out per sq-tile.  The next tile's
    first three AV groups carry graded due-slack so these copies stay off
    the PE critical path.  No PE transposes, no on-device normalize.
  - PSUM: psc 2x2 banks + po 2x1 + proj 2x1 = 8 banks.
  - startup: v projections run DURING the input DMA window (they only need
    xT+wv, which are ordered first); pair-0 q0/k0 follow; k1-3/q1-3 and all
    later pairs' projections are dripped into the chunk stream at <=2
    matmuls per iteration (bursts detour the PE FIFO and starve the ACT
    stream).  AV for chunk i is emitted AFTER scores for chunk i+1 (carried
    across tile/pair boundaries via a due-queue) so the strict-FIFO PE queue
    never head-of-line blocks the next ACT's input.

Steady state is TensorE-bound (~87% PE busy: 216ns/matmul warm, score pairs
~320ns for two concurrent K=64 matmuls incl. one exposed LDWEIGHTS); the
exp ACTIVATEs (1114ns per [128,1024] chunk) hide underneath.
"""

import numpy as np
import ml_dtypes
from contextlib import ExitStack

import concourse.bass as bass
import concourse.bacc as bacc
import concourse.mybir as mybir
import concourse.tile as tile
from concourse.bass_utils import run_bass_kernel_spmd

F32 = mybir.dt.float32
BF16 = mybir.dt.bfloat16

B, S, D = 4, 2048, 1024
H, DH = 16, 64
N_CORES = 8
HPC = 8          # heads per core
NPAIR = HPC // 2  # head pairs per core = 4
DPC = HPC * DH   # output cols per core = 512
SCALE = 1.0 / 32.0  # 1/sqrt(D)

KD = D // 128    # 8 contraction chunks over d_in
NS = S // 128    # 16 sk chunks
NT = S // 512    # 4 sq tiles of 512
OROWS = HPC * (DH + 1)  # 520 output rows (64 dh + denom per head)

_CACHE = {}


def _build_program():
    nc = bacc.Bacc("TRN2", target_bir_lowering=False, debug=False)

    xt_ext = nc.dram_tensor("xt", [D, S], BF16, kind="ExternalInput").ap()
    wq_ext = nc.dram_tensor("wq", [D, DPC], BF16, kind="ExternalInput").ap()
    wk_ext = nc.dram_tensor("wk", [D, DPC], BF16, kind="ExternalInput").ap()
    wv_ext = nc.dram_tensor("wv", [D, DPC], BF16, kind="ExternalInput").ap()
    bq_ext = nc.dram_tensor("bq", [DPC], F32, kind="ExternalInput").ap()
    bk_ext = nc.dram_tensor("bk", [DPC], F32, kind="ExternalInput").ap()
    out_ext = nc.dram_tensor("out", [OROWS, S], BF16, kind="ExternalOutput").ap()

    with tile.TileContext(nc, pool_alloc_mode="queue") as tc, ExitStack() as ctx:
        singles = ctx.enter_context(tc.tile_pool(name="singles", bufs=1))

        # --- DMAs: tiny bias vectors first, then x + v weights (v
        # projections overlap the DMA window), then q/k weights ---
        bq_col = []
        bk_col = []
        for m in range(NPAIR):
            t = singles.tile([128, 1], F32, tag=f"bq{m}", name=f"bq{m}")
            nc.sync.dma_start(
                out=t, in_=bq_ext[m * 128:(m + 1) * 128].rearrange("(p o) -> p o", o=1)
            )
            bq_col.append(t)
            t = singles.tile([128, 1], F32, tag=f"bk{m}", name=f"bk{m}")
            nc.scalar.dma_start(
                out=t, in_=bk_ext[m * 128:(m + 1) * 128].rearrange("(p o) -> p o", o=1)
            )
            bk_col.append(t)

        xT = [singles.tile([128, S], BF16, tag=f"xT{j}", name=f"xT{j}") for j in range(KD)]
        w_bf = {n: [None] * KD for n in ("wq", "wk", "wv")}

        def dma_w(name, ext, k, eng):
            wb = singles.tile([128, DPC], BF16, tag=f"{name}_bf{k}", name=f"{name}_bf{k}")
            eng.dma_start(out=wb, in_=ext[k * 128:(k + 1) * 128, :])
            w_bf[name][k] = wb

        for j in range(KD):
            # x chunk split across both HWDGE rings (SP + ACT) -- a single
            # ring's descriptor dispatch caps DMA throughput; wv rides along
            nc.sync.dma_start(out=xT[j][:, 0:S // 2], in_=xt_ext[j * 128:(j + 1) * 128, 0:S // 2])
            nc.scalar.dma_start(out=xT[j][:, S // 2:], in_=xt_ext[j * 128:(j + 1) * 128, S // 2:])
            dma_w("wv", wv_ext, j, nc.sync if j % 2 == 0 else nc.scalar)
        # wq/wk in two passes on the HW rings: only the pair-0 columns gate
        # the first scores, so they ride right after x+wv; the other three
        # pairs' columns (not read until pair-1's dripped projections,
        # ~60us in) follow once the critical window is over.  (A SWDGE
        # side-channel for the rest was tried and hurt: it steals HBM
        # bandwidth from the rings during the critical x window.)
        w_ext = {"wq": wq_ext, "wk": wk_ext}
        for j in range(KD):
            for name in ("wq", "wk"):
                wb = singles.tile([128, DPC], BF16, tag=f"{name}_bf{j}", name=f"{name}_bf{j}")
                w_bf[name][j] = wb
            nc.sync.dma_start(out=w_bf["wq"][j][:, 0:128], in_=wq_ext[j * 128:(j + 1) * 128, 0:128])
            nc.scalar.dma_start(out=w_bf["wk"][j][:, 0:128], in_=wk_ext[j * 128:(j + 1) * 128, 0:128])
        for j in range(KD):
            for i, name in enumerate(("wq", "wk")):
                eng = nc.sync if (j + i) % 2 == 0 else nc.scalar
                eng.dma_start(
                    out=w_bf[name][j][:, 128:DPC],
                    in_=w_ext[name][j * 128:(j + 1) * 128, 128:DPC],
                )

        # --- persistent sbuf tensors ---
        qT = [singles.tile([128, S], BF16, tag=f"qT{m}", name=f"qT{m}") for m in range(NPAIR)]
        kT = [singles.tile([128, S], BF16, tag=f"kT{m}", name=f"kT{m}") for m in range(NPAIR)]
        vsb = [singles.tile([128, HPC, DH + 1], BF16, tag=f"v{i}", name=f"v{i}") for i in range(NS)]
        outT = [singles.tile([DH + 1, S], BF16, tag=f"oT{h}", name=f"oT{h}") for h in range(HPC)]

        # preload the exp table set while DMAs stream
        dummy = singles.tile([1, 1], BF16, tag="dummy")
        nc.scalar.activation(
            dummy, bq_col[0][0:1, :], mybir.ActivationFunctionType.Exp, scale=SCALE
        )

        # dense warm-up matmuls on a zeroed tile with no DMA dependencies:
        # they bridge PE activity from ~1us until the first v-projection
        # matmuls (~14us) so the HAM clock gate reaches and keeps 2.4GHz --
        # without them the DMA-paced v trickle never fills the activity
        # window and the whole startup runs at 1.2GHz until ~33us.
        junk = singles.tile([128, 512], BF16, tag="junk")
        nc.vector.memset(junk, 0.0)

        # --- psum pools: scores 2x[128,1024]f32 (4 banks) + AV accumulators
        # 2x[65,512] (2 banks) + projection accumulators 2x[128,512] (2) ---
        s_psum = ctx.enter_context(tc.tile_pool(name="s_psum", bufs=2, space="PSUM"))
        o_psum = ctx.enter_context(tc.tile_pool(name="o_psum", bufs=2, space="PSUM"))
        p_psum = ctx.enter_context(tc.tile_pool(name="p_psum", bufs=2, space="PSUM"))

        def gen_1_proj(w, m, n, dst, bias):
            """Generator: one of q/k projection for pair m, sq quarter n."""
            sl = slice(n * 512, (n + 1) * 512)
            ps = p_psum.tile([128, 512], F32, tag="pp", name=f"pp{w}{m}_{n}")
            for k in range(KD):
                nc.tensor.matmul(
                    ps,
                    lhsT=w_bf[w][k][:, m * 128:(m + 1) * 128],
                    rhs=xT[k][:, sl],
                    start=(k == 0),
                    stop=(k == KD - 1),
                )
                if k % 2 == 1:
                    yield
            nc.vector.tensor_scalar_add(dst[m][:, sl], ps, bias[m])

        def gen_q_proj(m, n):
            return gen_1_proj("wq", m, n, qT, bq_col)

        def gen_k_proj(m, n):
            return gen_1_proj("wk", m, n, kT, bk_col)

        def emit(gen):
            for _ in gen:
                pass

        def gen_v_proj(i):
            """Generator: v projection for sk chunk i (no bias -- host adds
            bv after normalize since softmax rows sum to 1)."""
            ps = p_psum.tile([128, 512], F32, tag="pp", name=f"vp{i}")
            for k in range(KD):
                nc.tensor.matmul(
                    ps,
                    lhsT=xT[k][:, i * 128:(i + 1) * 128],
                    rhs=w_bf["wv"][k],
                    start=(k == 0),
                    stop=(k == KD - 1),
                )
                if k % 2 == 1:
                    yield
            nc.vector.tensor_copy(
                vsb[i][:, :, 0:DH], ps.rearrange("p (h d) -> p h d", h=HPC)
            )
            nc.vector.memset(vsb[i][:, :, DH:DH + 1], 1.0)

        warm = p_psum.tile([128, 512], F32, tag="pp", name="warm")
        for i in range(48):
            nc.tensor.matmul(
                warm, lhsT=junk[:, 0:128], rhs=junk, start=True, stop=True
            )

        # only the first two v projections and pair-0's first-quarter q/k
        # run before the chunk stream; v(2..15) and the other pair-0
        # quarters drip into tile 0's PE slack (the whole v stream before
        # the first score serialized ~20us of PE work behind the DMA tail)
        emit(gen_v_proj(0))
        emit(gen_v_proj(1))
        emit(gen_q_proj(0, 0))
        emit(gen_k_proj(0, 0))

        e_pool = ctx.enter_context(tc.tile_pool(name="e_pool", bufs=10))

        # filler[g] = list of steps (2 proj matmuls each) to emit at global
        # iteration g.  Deadlines: pair-0 k quarter n is needed by t=0
        # chunk c=4n; pair-0 q quarter n by tile t=n; pair p>0 fully by its
        # pair start (dripped uniformly over pair p-1's 64 iterations).
        n_glob = NPAIR * NT * NS
        filler = [[] for _ in range(n_glob + 1)]

        def schedule(gen, g0, g1, nsteps=5):
            """Spread `nsteps` consumption steps of gen over [g0, g1)."""
            slots = np.linspace(g0, g1, nsteps, endpoint=False).astype(int)
            for s in slots[:-1]:
                filler[s].append(gen.__next__)
            filler[slots[-1]].append(lambda gg=gen: list(gg))

        def atomic(gen, slot):
            """Emit a whole projection unit at one slot.  Tile-0 fillers
            must be atomic: interleaving two open generators plus a third
            allocation on the 2-buffer proj-psum tag would recycle a buffer
            mid-accumulation."""
            filler[slot].append(lambda gg=gen: list(gg))

        for c in range(2, NS):
            atomic(gen_v_proj(c), c - 2)
        atomic(gen_k_proj(0, 1), 0)
        atomic(gen_k_proj(0, 2), 3)
        atomic(gen_k_proj(0, 3), 6)
        atomic(gen_q_proj(0, 1), 10)
        # drip load-balancing: each pair drips its OWN tile-2/3 q quarters
        # early in its own span (deadlines are mid-pair), so the previous
        # pair's window only carries 6 donor units -- pair-0's tiles 1-3
        # were overloaded (~2.1 extra matmuls/iter, above the exp pace)
        # while pair-3 carried nothing
        units01 = [
            gen_q_proj(1, 0), gen_q_proj(0, 2), gen_k_proj(1, 0),
            gen_k_proj(1, 1), gen_q_proj(0, 3), gen_k_proj(1, 2),
            gen_k_proj(1, 3), gen_q_proj(1, 1),
        ]
        for u, g in enumerate(units01):
            schedule(g, 16 + u * 6, 16 + (u + 1) * 6, 5)
        for p in range(1, NPAIR):
            base2 = p * NT * NS
            schedule(gen_q_proj(p, 2), base2 + 2, base2 + 14, 5)
            schedule(gen_q_proj(p, 3), base2 + 14, base2 + 26, 5)
            if p + 1 < NPAIR:
                donors = [
                    gen_q_proj(p + 1, 0), gen_k_proj(p + 1, 0),
                    gen_k_proj(p + 1, 1), gen_k_proj(p + 1, 2),
                    gen_k_proj(p + 1, 3), gen_q_proj(p + 1, 1),
                ]
                for u, g in enumerate(donors):
                    schedule(g, base2 + 26 + u * 6, base2 + 26 + (u + 1) * 6, 5)

        # iterations are processed in groups of 2 chunks (fewer pending-
        # queue boundaries; the paired emission also keeps the psc pipeline
        # regular).  AVs trail their chunk by one group so the strict-FIFO
        # PE queue always has the next chunk's score matmuls ahead of any
        # instruction that waits on an exp result.
        pending = []  # (due_group, seq, fn): AVs, copies, DMAs
        seq_ctr = 0

        def push(due, fn):
            nonlocal seq_ctr
            pending.append((due, seq_ctr, fn))
            seq_ctr += 1

        def emit_exp(hp, t, c, psc):
            e = e_pool.tile([128, 1024], BF16, tag="e", name=f"e{hp}_{t}_{c}")
            nc.scalar.activation(
                e, psc, mybir.ActivationFunctionType.Exp, scale=SCALE
            )
            return e

        for g0 in range(0, n_glob, 2):
            G = g0 // 2
            hp, r = divmod(g0, NT * NS)
            t, c0 = divmod(r, NS)
            h0, h1 = 2 * hp, 2 * hp + 1
            tsl = slice(t * 512, (t + 1) * 512)
            if c0 == 0:
                po_h = o_psum.tile([DH + 1, 512], F32, tag="po", name=f"po{hp}_{t}a")
                po_h1 = o_psum.tile([DH + 1, 512], F32, tag="po", name=f"po{hp}_{t}b")
            pscs = []
            for c in (c0, c0 + 1):
                psc = s_psum.tile([128, 1024], F32, tag="psc", name=f"ps{hp}_{t}_{c}")
                pscs.append(psc)
                nc.tensor.matmul(
                    psc[:, 0:512],
                    lhsT=kT[hp][0:64, c * 128:(c + 1) * 128],
                    rhs=qT[hp][0:64, tsl],
                    start=True, stop=True,
                )
                nc.tensor.matmul(
                    psc[:, 512:1024],
                    lhsT=kT[hp][64:128, c * 128:(c + 1) * 128],
                    rhs=qT[hp][64:128, tsl],
                    start=True, stop=True,
                )
            due_now = sorted([p for p in pending if p[0] <= G], key=lambda p: p[1])
            pending = [p for p in pending if p[0] > G]
            for _, _, fn in due_now:
                fn()
            for g in (g0, g0 + 1):
                for fn in filler[g]:
                    fn()
                filler[g] = []
            for i, c in enumerate((c0, c0 + 1)):
                e = emit_exp(hp, t, c, pscs[i])

                def av(c=c, e=e, po_h=po_h, po_h1=po_h1, h0=h0, h1=h1):
                    nc.tensor.matmul(
                        po_h, lhsT=vsb[c][:, h0, :], rhs=e[:, 0:512],
                        start=(c == 0), stop=(c == NS - 1),
                    )
                    nc.tensor.matmul(
                        po_h1, lhsT=vsb[c][:, h1, :], rhs=e[:, 512:1024],
                        start=(c == 0), stop=(c == NS - 1),
                    )
                # the tile's first three AV groups get graded extra slack
                # (all landing at the same absolute group, so psum write
                # order stays c-ascending): the boundary po copies then
                # never sit on the PE's critical path
                gl = c0 // 2
                push(G + (3 if gl == 0 else 2 if gl == 1 else 1), av)
            if c0 + 1 == NS - 1:
                def tail(t=t, po_h=po_h, po_h1=po_h1, h0=h0, h1=h1):
                    # DVE-only copies: the deep AV slack above hides their
                    # latency, and keeping them off ScalarE avoids pausing
                    # the exp stream
                    tsl = slice(t * 512, (t + 1) * 512)
                    for h, po in ((h0, po_h), (h1, po_h1)):
                        nc.vector.tensor_copy(outT[h][:, tsl], po)
                        nc.sync.dma_start(
                            out=out_ext[h * (DH + 1):(h + 1) * (DH + 1), tsl],
                            in_=outT[h][:, tsl],
                        )
                push(G + 1, tail)
        for _, _, fn in sorted(pending, key=lambda p: p[1]):
            fn()
        for fns in filler:
            for fn in fns:
                fn()

    nc.compile()
    return nc


def _get_program():
    if "nc" not in _CACHE:
        _CACHE["nc"] = _build_program()
    return _CACHE["nc"]


def kernel(x, Wq, bq, Wk, bk, Wv, bv, _trace=False):
    bf = ml_dtypes.bfloat16
    x = np.asarray(x, dtype=np.float32)
    Wq = np.asarray(Wq, dtype=np.float32)
    Wk = np.asarray(Wk, dtype=np.float32)
    Wv = np.asarray(Wv, dtype=np.float32)
    bq = np.ascontiguousarray(np.asarray(bq, dtype=np.float32))
    bk = np.ascontiguousarray(np.asarray(bk, dtype=np.float32))
    bv = np.asarray(bv, dtype=np.float32)

    nc = _get_program()

    in_maps = []
    for c in range(N_CORES):
        b, g = c // 2, c % 2
        cols = slice(g * DPC, (g + 1) * DPC)
        in_maps.append(
            {
                "xt": np.ascontiguousarray(x[b].T.astype(bf)),
                "wq": np.ascontiguousarray(Wq[:, cols].astype(bf)),
                "wk": np.ascontiguousarray(Wk[:, cols].astype(bf)),
                "wv": np.ascontiguousarray(Wv[:, cols].astype(bf)),
                "bq": np.ascontiguousarray(bq[cols]),
                "bk": np.ascontiguousarray(bk[cols]),
            }
        )

    res = run_bass_kernel_spmd(nc, in_maps, core_ids=list(range(N_CORES)), trace=_trace)
    _CACHE["last_results"] = res

    out = np.empty((B, S, D), dtype=np.float32)
    for c in range(N_CORES):
        b, g = c // 2, c % 2
        o = res.results[c]["out"].astype(np.float32).reshape(HPC, DH + 1, S)
        nrm = o[:, 0:DH, :] / o[:, DH:DH + 1, :]      # [8, 64, S]
        out[b, :, g * DPC:(g + 1) * DPC] = nrm.transpose(2, 0, 1).reshape(S, DPC)
    out += bv
    return out
